# revision 27
# baseline (speedup 1.0000x reference)
"""Trainium2 Bass kernel for nn_CrossAttention_5385888989393.

Contract: kernel(**inputs) takes FULL inputs (batch 8) and returns the FULL
output, sharding batch-parallel across 8 NeuronCores (1 batch element per
core, no collectives).

Algorithm per batch (channel attention, contraction over spatial n=4096):
    G     = f_m @ f_n^T                     [512, 512]  Gram over n
    T2T   = G^T @ Wq^T                      [512, 512]  (G natural stationary)
    D^T_h = Wk_h-contraction with T2T       [64, 64] per head (diag tiles)
    E^T   = exp(D^T * scale) * headmask     (softmax numerator, transposed)
    SE_h  = E_h @ Wv_h   (via lhsT = E^T)   [64, 512]
    S_h   = SE_h / rowsum(E_h)              (deferred softmax normalization)
    M^T   = S-contraction with Wout^T       [512, 512]
    out   = (M @ f_n) + bout                [512, 4096]

~2x fewer FLOPs than the naive q/k/v path: the spatial dimension collapses
through the Gram matrix immediately.

Host-side marshalling does the heavy layout lifting so the device streams
big contiguous DMAs and runs back-to-back matmuls with nothing else on the
PE critical path:
  - f_m AND f_n are pre-TRANSPOSED on host to [n, c] tiles so both Gram
    operands come straight from DRAM -- zero PE transposes. f_n is shipped
    twice (transposed for phase 1, natural for phase 3); the extra 4.2 MiB
    of DMA streams during the phase-2 serial window where DMA is idle.
  - Everything is pre-cast to bf16 on host (validated 6.2e-3 rel err vs
    the 2e-2 gate): input DMA is ~12.6 MiB and no on-chip rounding casts.
    All matmuls run bf16 (1 cyc/row full-rate PE) with fp32 PSUM accum;
    output is fp32.
  - All tensors are packed [128, K] partition-major so every DMA kick is a
    plain 2D slice with >=1KB contiguous lines (the per-kick DIRECT2D cost
    on the sync engine is ~625ns; v1's 119 kicks nearly saturated it).
  - Weight kicks are woven into the tail of the data stream (after chunks
    6/7) so they land just before the phase-2 chain needs them; the f_n
    natural copy streams last, during the phase-2 window, ahead of phase 3.
  - The Gram DMA stream leads the PE by chunk-0..2 half-kicks, 4-deep pool
    buffering, and a late const kick, keeping phase 1 stall-free; output
    stores drain via one kick per [128,512] tile with a pair+singles split
    on the final chunk to shorten the tail.
"""
import sys

if "/opt/trn_rl_repo" not in sys.path:
    sys.path.insert(0, "/opt/trn_rl_repo")

import numpy as np
import ml_dtypes

import concourse.bass as bass
import concourse.tile as tile
from concourse import bacc, mybir
from concourse.bass_utils import run_bass_kernel_spmd

F32 = mybir.dt.float32
BF16 = mybir.dt.bfloat16
EXP = mybir.ActivationFunctionType.Exp
CP = mybir.ActivationFunctionType.Copy
IDENT_FN = mybir.ActivationFunctionType.Identity

P = 128          # partitions
C = 512          # channels
CT = C // P      # 4 channel tiles
NN = 4096        # spatial (64*64)
NCH = NN // 512  # 8 column chunks of 512
BLK = 2048       # per-chunk free elements per partition (4 tiles x 512)
DH = 64
SCALE = DH ** -0.5
B = 8            # batch == n_cores
NSUB = 32        # total 128-row subchunks of n

_BF = ml_dtypes.bfloat16

_CACHED_NC = None
_CACHED_RUNNER = None


def _host_consts():
    # dmask_wide: per-head-pair mask replicated for the 4 stacked jt tiles
    dm = np.kron(np.eye(2, dtype=np.float32), np.ones((DH, DH), np.float32))
    return np.tile(dm, (1, 4))


_DMASKW = _host_consts()


def _build():
    nc = bacc.Bacc("TRN2", target_bir_lowering=False, debug=False, num_devices=B)

    fm_d = nc.dram_tensor("fm2", [P, NCH * BLK], BF16, kind="ExternalInput").ap()
    fnt_d = nc.dram_tensor("fnt2", [P, NCH * BLK], BF16, kind="ExternalInput").ap()
    fn_d = nc.dram_tensor("fn2", [P, NCH * BLK], BF16, kind="ExternalInput").ap()
    w_d = nc.dram_tensor("w2", [P, 16 * C], BF16, kind="ExternalInput").ap()
    cstf_d = nc.dram_tensor("cstf", [P, 516], F32, kind="ExternalInput").ap()
    out_d = nc.dram_tensor("out", [P, NCH * BLK], F32, kind="ExternalOutput").ap()

    with tile.TileContext(nc) as tc:
        with (
            tc.tile_pool(name="const", bufs=1) as const,
            tc.tile_pool(name="wall", bufs=1) as wallp,
            tc.tile_pool(name="fm", bufs=4) as fmpool,
            tc.tile_pool(name="fnt", bufs=4) as fntpool,
            tc.tile_pool(name="fn", bufs=1) as fnpool,
            tc.tile_pool(name="small", bufs=1) as small,
            tc.tile_pool(name="outst", bufs=2) as outst,
            tc.tile_pool(name="gacc", bufs=1, space="PSUM") as gacc,
            tc.tile_pool(name="work", bufs=2, space="PSUM") as work,
        ):
            # ---------- startup ----------
            # chunk 0-2 data is split into half-kicks so the first Gram
            # matmuls start as early as possible and the early chunk
            # pipeline never starves; cstf (mask+bias, needed only in
            # phase 2) goes after chunk 0.
            fnt0a = const.tile([P, BLK // 2], BF16, tag="fnt0a")
            nc.sync.dma_start(fnt0a[:], fnt_d[:, 0:BLK // 2])
            fm0a = const.tile([P, BLK // 2], BF16, tag="fm0a")
            nc.sync.dma_start(fm0a[:], fm_d[:, 0:BLK // 2])
            fnt0b = const.tile([P, BLK // 2], BF16, tag="fnt0b")
            nc.sync.dma_start(fnt0b[:], fnt_d[:, BLK // 2:BLK])
            fm0b = const.tile([P, BLK // 2], BF16, tag="fm0b")
            nc.sync.dma_start(fm0b[:], fm_d[:, BLK // 2:BLK])
            ch0 = [
                (fnt0a, fm0a, 0), (fnt0a, fm0a, 512),
                (fnt0b, fm0b, 0), (fnt0b, fm0b, 512),
            ]
            cstf = const.tile([P, 516], F32, tag="cstf")
            dmaskw = cstf[:, 0:512]
            bout_sb = [cstf[:, 512 + ct:513 + ct] for ct in range(CT)]

            # HAM warm-up: back-to-back transposes of a memset-zero tile
            # (no DMA dependency -> starts immediately) fill the PE-idle
            # startup window with sustained PE activity so the first real
            # matmuls run at full clock. The chain stays transitively live
            # because the rowsum ones-vector is produced from the (zero)
            # warm-up output via a +1.0 bias. Slice 0 is written once
            # (rep 0) so the ones2w read has no WAR hazard with later reps.
            zt0 = const.tile([P, P], BF16, tag="zt0")
            nc.gpsimd.memset(zt0[:], 0.0)
            warm_ps = work.tile([P, C], BF16, tag="wk0", name="warmps")
            ones2w = const.tile([P, 2], BF16, tag="ones2w")
            nc.tensor.transpose(warm_ps[:, 0:P], zt0[:], zt0[:])
            for i in range(24):
                wsl = slice((i % 3 + 1) * P, (i % 3 + 2) * P)
                nc.tensor.transpose(warm_ps[:, wsl], zt0[:], zt0[:])
            nc.scalar.activation(ones2w[:], warm_ps[:, 0:2], CP, bias=1.0)

            # ---------- phase 1: Gram accumulation over 32 subchunks ------
            # Both operands arrive pre-transposed [n-part, c]; pure matmuls.
            g_ps = [
                gacc.tile([P, C], F32, tag=f"g{at}", name=f"g_ps{at}")
                for at in range(CT)
            ]
            wall = wallp.tile([P, 16 * C], BF16, tag="w")
            for su in range(4):
                fnt_t, fm_t, off = ch0[su]
                for at in range(CT):
                    nc.tensor.matmul(
                        g_ps[at][:],
                        fm_t[:, off + at * P: off + (at + 1) * P],
                        fnt_t[:, off:off + 512],
                        start=(su == 0),
                        stop=False,
                    )
            halves = {}
            for ch in (1, 2):
                for hf in range(2):
                    o = ch * BLK + hf * (BLK // 2)
                    t = fntpool.tile([P, BLK // 2], BF16, tag="fnth")
                    nc.sync.dma_start(t[:], fnt_d[:, o:o + BLK // 2])
                    halves[("fnt", ch, hf)] = t
                    t = fmpool.tile([P, BLK // 2], BF16, tag="fmh")
                    nc.sync.dma_start(t[:], fm_d[:, o:o + BLK // 2])
                    halves[("fm", ch, hf)] = t
            G_sb = [
                small.tile([P, C], BF16, tag=f"G{at}", name=f"G_sb{at}")
                for at in range(CT)
            ]
            for ch in range(1, NCH):
                if ch > 2:
                    fntr = fntpool.tile([P, BLK], BF16, tag="fnt")
                    nc.sync.dma_start(fntr[:], fnt_d[:, ch * BLK:(ch + 1) * BLK])
                    fmr = fmpool.tile([P, BLK], BF16, tag="fm")
                    nc.sync.dma_start(fmr[:], fm_d[:, ch * BLK:(ch + 1) * BLK])
                # weight kicks woven into the stream so WqT/WkT land just
                # before the phase-2 chain needs them without delaying the
                # last data chunks
                if ch == 5:
                    # mask+bias consts: needed only at EXP time; kicked here
                    # so chunks 1-4 get maximal DMA lead over the PE
                    nc.sync.dma_start(cstf[:], cstf_d)
                elif ch == 6:
                    nc.sync.dma_start(wall[:, 0:4 * C], w_d[:, 0:4 * C])
                elif ch == 7:
                    nc.sync.dma_start(wall[:, 4 * C:8 * C], w_d[:, 4 * C:8 * C])
                for su in range(4):
                    s = ch * 4 + su
                    if ch <= 2:
                        fnt_t = halves[("fnt", ch, su // 2)]
                        fm_t = halves[("fm", ch, su // 2)]
                        off = (su % 2) * 512
                    else:
                        fnt_t, fm_t, off = fntr, fmr, su * 512
                    for at in range(CT):
                        nc.tensor.matmul(
                            g_ps[at][:],
                            fm_t[:, off + at * P: off + (at + 1) * P],
                            fnt_t[:, off:off + 512],
                            start=False,
                            stop=(s == NSUB - 1),
                        )

            # remaining weights (Wv, WoutT) right after the last data chunk
            nc.sync.dma_start(wall[:, 8 * C:16 * C], w_d[:, 8 * C:16 * C])
            WqT = [wall[:, (0 * CT + t) * C:(0 * CT + t + 1) * C] for t in range(CT)]
            WkT = [wall[:, (1 * CT + t) * C:(1 * CT + t + 1) * C] for t in range(CT)]
            Wv = [wall[:, (2 * CT + t) * C:(2 * CT + t + 1) * C] for t in range(CT)]
            WoT = [wall[:, (3 * CT + t) * C:(3 * CT + t + 1) * C] for t in range(CT)]

            # f_n natural layout for phase 3: streams in during phase 2
            fnr_res = []
            for ch in range(NCH):
                fnr = fnpool.tile([P, BLK], BF16, tag=f"fn{ch}", name=f"fn{ch}")
                nc.sync.dma_start(fnr[:], fn_d[:, ch * BLK:(ch + 1) * BLK])
                fnr_res.append(fnr)

            # ---------- phase 2: logits, softmax, value mixing ------------
            for at in range(CT):
                if at % 2 == 0:
                    nc.vector.tensor_copy(G_sb[at][:], g_ps[at][:])
                else:
                    nc.scalar.activation(G_sb[at][:], g_ps[at][:], CP)

            # T2T[b, (h,i)] = sum_a G[a, b] * WqT[a, (h,i)]
            T2T_sb = []
            for bt in range(CT):
                ps = work.tile([P, C], F32, tag="wk1", name="t2tps")
                for at in range(CT):
                    nc.tensor.matmul(
                        ps[:],
                        G_sb[at][:, bt * P:(bt + 1) * P],
                        WqT[at][:],
                        start=(at == 0),
                        stop=(at == CT - 1),
                    )
                t = small.tile([P, C], BF16, tag=f"T2T_{bt}")
                if bt % 2 == 0:
                    nc.vector.tensor_copy(t[:], ps[:])
                else:
                    nc.scalar.activation(t[:], ps[:], CP)
                T2T_sb.append(t)

            # Diagonal head-pair tiles of D^T, stacked into two [128, 256]
            # PSUM tiles (split so the first half's exp doesn't wait on the
            # second half's matmuls -- psum deps are tile-granular)
            dps = [
                work.tile([P, 256], F32, tag="wk0", name=f"dps{hf}")
                for hf in range(2)
            ]
            for jt in range(CT):
                sl = slice(jt * P, (jt + 1) * P)
                dsl = slice((jt % 2) * P, (jt % 2 + 1) * P)
                for bt in range(CT):
                    nc.tensor.matmul(
                        dps[jt // 2][:, dsl], WkT[bt][:, sl], T2T_sb[bt][:, sl],
                        start=(bt == 0), stop=(bt == CT - 1),
                    )
            # E^T = exp(scale * D^T), cross-head blocks zeroed; per-half
            # exp/mask/rowsum/recip pipelines against the other half
            ew = small.tile([P, C], BF16, tag="ew")
            invw = small.tile([P, 8], F32, tag="invw")
            for hf in range(2):
                hs = slice(hf * 256, (hf + 1) * 256)
                etmp = small.tile([P, 256], F32, tag=f"etmp{hf}")
                nc.scalar.activation(etmp[:], dps[hf][:], EXP, scale=SCALE)
                nc.vector.tensor_mul(ew[:, hs], etmp[:], dmaskw[:, hs])
                rps = work.tile([P, 4], F32, tag="wk1", name=f"rps{hf}")
                for it in (2 * hf, 2 * hf + 1):
                    nc.tensor.matmul(rps[:, (it % 2) * 2:(it % 2) * 2 + 2],
                                     ew[:, it * P:(it + 1) * P], ones2w[:],
                                     start=True, stop=True)
                nc.vector.reciprocal(invw[:, hf * 4:(hf + 1) * 4], rps[:])

            # SE_h = E_h @ Wv_h. The softmax division is folded into Wout
            # instead of SE (rows of Wout^T are e=(h,i)-indexed, same as SE
            # rows): M = sum_e SE[e,:] * inv[e] * WoT[e,:] -- the row-scale
            # of WoT runs parallel to the SE casts on the other engine.
            SE_sb = []
            WoTs = []
            for it in range(CT):
                seps = gacc.tile([P, C], F32, tag=f"g{it}", name=f"seps{it}")
                nc.tensor.matmul(seps[:], ew[:, it * P:(it + 1) * P],
                                 Wv[it][:], start=True, stop=True)
                se_t = small.tile([P, C], BF16, tag=f"SE{it}")
                ws_t = small.tile([P, C], BF16, tag=f"WoTs{it}")
                inv_ap = invw[:, it * 2:it * 2 + 1]
                if it % 2 == 0:
                    nc.vector.tensor_copy(se_t[:], seps[:])
                else:
                    nc.scalar.activation(se_t[:], seps[:], CP)
                # WoTs runs on the otherwise-idle GPSIMD engine (SBUF->SBUF)
                # in parallel with the SE casts on DVE/ACT
                nc.gpsimd.tensor_scalar_mul(ws_t[:], WoT[it][:], inv_ap)
                SE_sb.append(se_t)
                WoTs.append(ws_t)

            # M^T[c, o] = sum_e SE[e][:, c] * WoTs[e][:, o]; et-major so each
            # accumulation step runs as soon as its SE/WoTs pair is ready
            mt_ps = [
                gacc.tile([P, C], F32, tag=f"g{ct}", name=f"mtps{ct}")
                for ct in range(CT)
            ]
            for et in range(CT):
                for ct in range(CT):
                    nc.tensor.matmul(
                        mt_ps[ct][:],
                        SE_sb[et][:, ct * P:(ct + 1) * P],
                        WoTs[et][:],
                        start=(et == 0),
                        stop=(et == CT - 1),
                    )
            MT_sb = []
            for ct in range(CT):
                t = small.tile([P, C], BF16, tag=f"MT{ct}")
                if ct % 2 == 0:
                    nc.vector.tensor_copy(t[:], mt_ps[ct][:])
                else:
                    nc.scalar.activation(t[:], mt_ps[ct][:], CP)
                MT_sb.append(t)

            # ---------- phase 3: out = M @ f_n + bout ----------------------
            # per-[128,512]-tile bias-add + store so the out stream drains
            # continuously (DVE/ACT alternate)
            # out kicks cost ~650ns each serialized on the sync engine, so
            # chunks 0-6 use one kick per ot-PAIR (2/chunk); the last chunk
            # uses pair + two singles so the final add->kick->transfer chain
            # after the very last matmul is as short as possible
            for ch in range(NCH):
                fnr = fnr_res[ch]
                last = ch == NCH - 1
                opair = None
                for ot in range(CT):
                    ps = gacc.tile([P, 512], F32, tag=f"g{ot}", name=f"ops{ot}")
                    for ct in range(CT):
                        nc.tensor.matmul(
                            ps[:],
                            MT_sb[ct][:, ot * P:(ot + 1) * P],
                            fnr[:, ct * 512:(ct + 1) * 512],
                            start=(ct == 0),
                            stop=(ct == CT - 1),
                        )
                    base = ch * BLK + ot * 512
                    if not last:
                        o = outst.tile([P, 512], F32, tag=f"o{ot}")
                        if ot % 2 == 0:
                            nc.vector.tensor_scalar_add(o[:], ps[:],
                                                        bout_sb[ot])
                        else:
                            nc.scalar.activation(o[:], ps[:], IDENT_FN,
                                                 bias=bout_sb[ot])
                        nc.sync.dma_start(out_d[:, base:base + 512], o[:])
                    elif ot < 2:
                        if ot == 0:
                            opair = outst.tile([P, 1024], F32, tag="opL")
                        osl = opair[:, ot * 512:(ot + 1) * 512]
                        if ot % 2 == 0:
                            nc.vector.tensor_scalar_add(osl, ps[:],
                                                        bout_sb[ot])
                        else:
                            nc.scalar.activation(osl, ps[:], IDENT_FN,
                                                 bias=bout_sb[ot])
                        if ot == 1:
                            nc.sync.dma_start(
                                out_d[:, base - 512:base + 512], opair[:]
                            )
                    else:
                        o = outst.tile([P, 512], F32, tag=f"oL{ot}")
                        if ot == 3:
                            # very last tile: halves on both engines in
                            # parallel to shorten the drain chain
                            nc.vector.tensor_scalar_add(o[:, 0:256],
                                                        ps[:, 0:256],
                                                        bout_sb[ot])
                            nc.scalar.activation(o[:, 256:512],
                                                 ps[:, 256:512], IDENT_FN,
                                                 bias=bout_sb[ot])
                        else:
                            nc.vector.tensor_scalar_add(o[:], ps[:],
                                                        bout_sb[ot])
                        nc.sync.dma_start(out_d[:, base:base + 512], o[:])

    nc.compile()
    return nc


def _get_nc():
    global _CACHED_NC
    if _CACHED_NC is None:
        _CACHED_NC = _build()
    return _CACHED_NC


def _get_runner():
    """Memoized PJRT runner: jax.jit-compiled once, reused across kernel()
    calls (run_bass_kernel_spmd rebuilds the jit closure every call, which
    forces a ~minute-long recompile)."""
    global _CACHED_RUNNER
    if _CACHED_RUNNER is not None:
        return _CACHED_RUNNER

    import jax
    from jax.sharding import Mesh, PartitionSpec
    from jax.experimental.shard_map import shard_map
    import concourse.mybir as mybir_
    from concourse.bass2jax import (
        _bass_exec_p,
        install_neuronx_cc_hook,
        partition_id_tensor,
    )

    nc = _get_nc()
    install_neuronx_cc_hook()

    partition_name = (
        nc.partition_id_tensor.name if nc.partition_id_tensor else None
    )
    in_names = []
    out_names = []
    out_avals = []
    out_shapes = []
    for alloc in nc.m.functions[0].allocations:
        if not isinstance(alloc, mybir_.MemoryLocationSet):
            continue
        name = alloc.memorylocations[0].name
        if alloc.kind == "ExternalInput":
            if name != partition_name:
                in_names.append(name)
        elif alloc.kind == "ExternalOutput":
            shape = tuple(alloc.tensor_shape)
            dtype = mybir_.dt.np(alloc.dtype)
            out_names.append(name)
            out_avals.append(jax.core.ShapedArray(shape, dtype))
            out_shapes.append((shape, dtype))
    n_params = len(in_names)
    n_outs = len(out_names)
    all_names = tuple(in_names + out_names)
    if partition_name is not None:
        all_names = all_names + (partition_name,)
    donate = tuple(range(n_params, n_params + n_outs))

    def _body(*args):
        operands = list(args)
        if partition_name is not None:
            operands.append(partition_id_tensor())
        outs = _bass_exec_p.bind(
            *operands,
            out_avals=tuple(out_avals),
            in_names=all_names,
            out_names=tuple(out_names),
            lowering_input_output_aliases=(),
            sim_require_finite=True,
            sim_require_nnan=True,
            nc=nc,
        )
        return tuple(outs)

    devices = jax.devices()[:B]
    mesh = Mesh(np.asarray(devices), ("core",))
    sharded = jax.jit(
        shard_map(
            _body,
            mesh=mesh,
            in_specs=(PartitionSpec("core"),) * (n_params + n_outs),
            out_specs=(PartitionSpec("core"),) * n_outs,
            check_rep=False,
        ),
        donate_argnums=donate,
        keep_unused=True,
    )

    def run(in_maps):
        concat_in = [
            np.concatenate([np.asarray(m[k]) for m in in_maps], axis=0)
            for k in in_names
        ]
        concat_zeros = [
            np.zeros((B * s[0], *s[1:]), dt) for (s, dt) in out_shapes
        ]
        out_arrs = sharded(*concat_in, *concat_zeros)
        return [
            {
                k: np.asarray(out_arrs[i]).reshape(B, *out_shapes[i][0])[c]
                for i, k in enumerate(out_names)
            }
            for c in range(B)
        ]

    _CACHED_RUNNER = run
    return run


def _marshal(f_m, f_n, Wq, Wkv, Wout, bout):
    f32 = np.float32
    b = f_m.shape[0]
    # x2[p, (ch, su, c)] = x[c, (ch*4+su)*128 + p]  (pre-transposed [n, c])
    fm = np.asarray(f_m, f32).reshape(b, C, NSUB, P)
    fm2 = np.ascontiguousarray(fm.transpose(0, 3, 2, 1).reshape(b, P, NCH * BLK))
    fm2 = fm2.astype(_BF)
    fnt = np.asarray(f_n, f32).reshape(b, C, NSUB, P)
    fnt2 = np.ascontiguousarray(fnt.transpose(0, 3, 2, 1).reshape(b, P, NCH * BLK))
    fnt2 = fnt2.astype(_BF)
    # fn2[p, (ch, ct, n')] = f_n[ct*128 + p, ch*512 + n']  (natural)
    fn = np.asarray(f_n, f32).reshape(b, CT, P, NCH, 512)
    fn2 = np.ascontiguousarray(fn.transpose(0, 2, 3, 1, 4).reshape(b, P, NCH * BLK))
    fn2 = fn2.astype(_BF)

    Wq = np.asarray(Wq, f32)
    Wkv = np.asarray(Wkv, f32)
    Wout = np.asarray(Wout, f32)
    w2 = np.empty((P, 16 * C), f32)
    for wi, W in enumerate([Wq.T, Wkv[:C].T, Wkv[C:], Wout.T]):
        for t in range(CT):
            w2[:, (wi * CT + t) * C:(wi * CT + t + 1) * C] = W[t * P:(t + 1) * P, :]
    w2 = w2.astype(_BF)

    bout = np.asarray(bout, f32)
    cstf = np.zeros((P, 516), f32)
    cstf[:, 0:512] = _DMASKW
    cstf[:, 512:516] = bout.reshape(CT, P).T
    return fm2, fnt2, fn2, w2, cstf


def kernel(f_m, f_n, Wq, Wkv, Wout, bout, trace=False):
    b, c, h, w = f_m.shape
    nc = _get_nc()
    fm2, fnt2, fn2, w2, cstf = _marshal(f_m, f_n, Wq, Wkv, Wout, bout)
    in_maps = [
        {
            "fm2": fm2[i],
            "fnt2": fnt2[i],
            "fn2": fn2[i],
            "w2": w2,
            "cstf": cstf,
        }
        for i in range(b)
    ]
    if trace:
        res = run_bass_kernel_spmd(
            nc, in_maps, core_ids=list(range(B)), trace=True
        )
        kernel.last_results = res
        results = res.results
    else:
        results = _get_runner()(in_maps)
    # out[p, (ch, ot, n')] -> out_full[ot*128+p, ch*512+n']
    outs = []
    for r in results:
        o = r["out"].reshape(P, NCH, CT, 512)
        outs.append(
            np.ascontiguousarray(o.transpose(2, 0, 1, 3)).reshape(c, h, w)
        )
    return np.stack(outs)


# revision 28
# speedup vs baseline: 1.3350x; 1.3350x over previous
"""Trainium2 Bass kernel for nn_CrossAttention_5385888989393.

Contract: kernel(**inputs) takes FULL inputs (batch 8) and returns the FULL
output, sharding batch-parallel across 8 NeuronCores (1 batch element per
core, no collectives).

Algorithm per batch (channel attention, contraction over spatial n=4096):
    G     = f_m @ f_n^T                     [512, 512]  Gram over n
    T2T   = G^T @ Wq^T                      [512, 512]  (G natural stationary)
    D^T_h = Wk_h-contraction with T2T       [64, 64] per head (diag tiles)
    E^T   = exp(D^T * scale) * headmask     (softmax numerator, transposed)
    SE_h  = E_h @ Wv_h   (via lhsT = E^T)   [64, 512]
    S_h   = SE_h / rowsum(E_h)              (deferred softmax normalization)
    M^T   = S-contraction with Wout^T       [512, 512]
    out   = (M @ f_n) + bout                [512, 4096]

~2x fewer FLOPs than the naive q/k/v path: the spatial dimension collapses
through the Gram matrix immediately.

Host-side marshalling does the heavy layout lifting so the device streams
big contiguous DMAs and runs back-to-back matmuls with nothing else on the
PE critical path:
  - f_m AND f_n are pre-TRANSPOSED on host to [n, c] tiles so both Gram
    operands come straight from DRAM -- zero PE transposes. f_n is shipped
    twice (transposed for phase 1, natural for phase 3); the extra 4.2 MiB
    of DMA streams during the phase-2 serial window where DMA is idle.
  - Everything is pre-cast to bf16 on host (validated 6.2e-3 rel err vs
    the 2e-2 gate): input DMA is ~12.6 MiB and no on-chip rounding casts.
    All matmuls run bf16 (1 cyc/row full-rate PE) with fp32 PSUM accum;
    output is fp32.
  - All tensors are packed [128, K] partition-major so every DMA kick is a
    plain 2D slice with >=1KB contiguous lines (the per-kick DIRECT2D cost
    on the sync engine is ~625ns; v1's 119 kicks nearly saturated it).
  - Weight kicks are woven into the tail of the data stream (after chunks
    6/7) so they land just before the phase-2 chain needs them; the f_n
    natural copy streams last, during the phase-2 window, ahead of phase 3.
  - The Gram DMA stream leads the PE by chunk-0..2 half-kicks, 4-deep pool
    buffering, and a late const kick, keeping phase 1 stall-free; output
    stores drain via one kick per [128,512] tile with a pair+singles split
    on the final chunk to shorten the tail.
"""
import sys

if "/opt/trn_rl_repo" not in sys.path:
    sys.path.insert(0, "/opt/trn_rl_repo")

import numpy as np
import ml_dtypes

import concourse.bass as bass
import concourse.tile as tile
from concourse import bacc, mybir
from concourse.bass_utils import run_bass_kernel_spmd

F32 = mybir.dt.float32
BF16 = mybir.dt.bfloat16
EXP = mybir.ActivationFunctionType.Exp
CP = mybir.ActivationFunctionType.Copy
IDENT_FN = mybir.ActivationFunctionType.Identity

P = 128          # partitions
C = 512          # channels
CT = C // P      # 4 channel tiles
NN = 4096        # spatial (64*64)
NCH = NN // 512  # 8 column chunks of 512
BLK = 2048       # per-chunk free elements per partition (4 tiles x 512)
DH = 64
SCALE = DH ** -0.5
B = 8            # batch == n_cores
NSUB = 32        # total 128-row subchunks of n

_BF = ml_dtypes.bfloat16

_CACHED_NC = None
_CACHED_RUNNER = None


def _host_consts():
    # dmask_wide: per-head-pair mask replicated for the 4 stacked jt tiles
    dm = np.kron(np.eye(2, dtype=np.float32), np.ones((DH, DH), np.float32))
    return np.tile(dm, (1, 4))


_DMASKW = _host_consts()


def _build():
    nc = bacc.Bacc("TRN2", target_bir_lowering=False, debug=False, num_devices=B)

    fm_d = nc.dram_tensor("fm2", [P, NCH * BLK], BF16, kind="ExternalInput").ap()
    fnt_d = nc.dram_tensor("fnt2", [P, NCH * BLK], BF16, kind="ExternalInput").ap()
    fn_d = nc.dram_tensor("fn2", [P, NCH * BLK], BF16, kind="ExternalInput").ap()
    w_d = nc.dram_tensor("w2", [P, 16 * C], BF16, kind="ExternalInput").ap()
    cstf_d = nc.dram_tensor("cstf", [P, 516], F32, kind="ExternalInput").ap()
    out_d = nc.dram_tensor("out", [P, NCH * BLK], F32, kind="ExternalOutput").ap()

    with tile.TileContext(nc) as tc:
        with (
            tc.tile_pool(name="const", bufs=1) as const,
            tc.tile_pool(name="wall", bufs=1) as wallp,
            tc.tile_pool(name="fm", bufs=4) as fmpool,
            tc.tile_pool(name="fnt", bufs=4) as fntpool,
            tc.tile_pool(name="fn", bufs=1) as fnpool,
            tc.tile_pool(name="small", bufs=1) as small,
            tc.tile_pool(name="outst", bufs=2) as outst,
            tc.tile_pool(name="gacc", bufs=1, space="PSUM") as gacc,
            tc.tile_pool(name="work", bufs=2, space="PSUM") as work,
        ):
            # ---------- startup ----------
            # chunk 0-2 data is split into half-kicks so the first Gram
            # matmuls start as early as possible and the early chunk
            # pipeline never starves; cstf (mask+bias, needed only in
            # phase 2) goes after chunk 0.
            fnt0a = const.tile([P, BLK // 2], BF16, tag="fnt0a")
            nc.sync.dma_start(fnt0a[:], fnt_d[:, 0:BLK // 2])
            fm0a = const.tile([P, BLK // 2], BF16, tag="fm0a")
            nc.sync.dma_start(fm0a[:], fm_d[:, 0:BLK // 2])
            fnt0b = const.tile([P, BLK // 2], BF16, tag="fnt0b")
            nc.sync.dma_start(fnt0b[:], fnt_d[:, BLK // 2:BLK])
            fm0b = const.tile([P, BLK // 2], BF16, tag="fm0b")
            nc.sync.dma_start(fm0b[:], fm_d[:, BLK // 2:BLK])
            ch0 = [
                (fnt0a, fm0a, 0), (fnt0a, fm0a, 512),
                (fnt0b, fm0b, 0), (fnt0b, fm0b, 512),
            ]
            cstf = const.tile([P, 516], F32, tag="cstf")
            dmaskw = cstf[:, 0:512]
            bout_sb = [cstf[:, 512 + ct:513 + ct] for ct in range(CT)]

            # HAM warm-up: back-to-back transposes of a memset-zero tile
            # (no DMA dependency -> starts immediately) fill the PE-idle
            # startup window with sustained PE activity so the first real
            # matmuls run at full clock. The chain stays transitively live
            # because the rowsum ones-vector is produced from the (zero)
            # warm-up output via a +1.0 bias. Slice 0 is written once
            # (rep 0) so the ones2w read has no WAR hazard with later reps.
            zt0 = const.tile([P, P], BF16, tag="zt0")
            nc.gpsimd.memset(zt0[:], 0.0)
            warm_ps = work.tile([P, C], BF16, tag="wk0", name="warmps")
            ones2w = const.tile([P, 2], BF16, tag="ones2w")
            nc.tensor.transpose(warm_ps[:, 0:P], zt0[:], zt0[:])
            for i in range(24):
                wsl = slice((i % 3 + 1) * P, (i % 3 + 2) * P)
                nc.tensor.transpose(warm_ps[:, wsl], zt0[:], zt0[:])
            nc.scalar.activation(ones2w[:], warm_ps[:, 0:2], CP, bias=1.0)

            # ---------- phase 1: Gram accumulation over 32 subchunks ------
            # Both operands arrive pre-transposed [n-part, c]; pure matmuls.
            g_ps = [
                gacc.tile([P, C], F32, tag=f"g{at}", name=f"g_ps{at}")
                for at in range(CT)
            ]
            wall = wallp.tile([P, 16 * C], BF16, tag="w")
            for su in range(4):
                fnt_t, fm_t, off = ch0[su]
                for at in range(CT):
                    nc.tensor.matmul(
                        g_ps[at][:],
                        fm_t[:, off + at * P: off + (at + 1) * P],
                        fnt_t[:, off:off + 512],
                        start=(su == 0),
                        stop=False,
                    )
            halves = {}
            for ch in (1, 2):
                for hf in range(2):
                    o = ch * BLK + hf * (BLK // 2)
                    t = fntpool.tile([P, BLK // 2], BF16, tag="fnth")
                    nc.sync.dma_start(t[:], fnt_d[:, o:o + BLK // 2])
                    halves[("fnt", ch, hf)] = t
                    t = fmpool.tile([P, BLK // 2], BF16, tag="fmh")
                    nc.sync.dma_start(t[:], fm_d[:, o:o + BLK // 2])
                    halves[("fm", ch, hf)] = t
            G_sb = [
                small.tile([P, C], BF16, tag=f"G{at}", name=f"G_sb{at}")
                for at in range(CT)
            ]
            for ch in range(1, NCH):
                if ch > 2:
                    fntr = fntpool.tile([P, BLK], BF16, tag="fnt")
                    nc.sync.dma_start(fntr[:], fnt_d[:, ch * BLK:(ch + 1) * BLK])
                    fmr = fmpool.tile([P, BLK], BF16, tag="fm")
                    nc.sync.dma_start(fmr[:], fm_d[:, ch * BLK:(ch + 1) * BLK])
                # weight kicks woven into the stream so WqT/WkT land just
                # before the phase-2 chain needs them without delaying the
                # last data chunks
                if ch == 5:
                    # mask+bias consts: needed only at EXP time; kicked here
                    # so chunks 1-4 get maximal DMA lead over the PE
                    nc.sync.dma_start(cstf[:], cstf_d)
                elif ch == 6:
                    nc.sync.dma_start(wall[:, 0:4 * C], w_d[:, 0:4 * C])
                elif ch == 7:
                    nc.sync.dma_start(wall[:, 4 * C:8 * C], w_d[:, 4 * C:8 * C])
                for su in range(4):
                    s = ch * 4 + su
                    if ch <= 2:
                        fnt_t = halves[("fnt", ch, su // 2)]
                        fm_t = halves[("fm", ch, su // 2)]
                        off = (su % 2) * 512
                    else:
                        fnt_t, fm_t, off = fntr, fmr, su * 512
                    for at in range(CT):
                        nc.tensor.matmul(
                            g_ps[at][:],
                            fm_t[:, off + at * P: off + (at + 1) * P],
                            fnt_t[:, off:off + 512],
                            start=False,
                            stop=(s == NSUB - 1),
                        )

            # remaining weights (Wv, WoutT) right after the last data chunk
            nc.sync.dma_start(wall[:, 8 * C:16 * C], w_d[:, 8 * C:16 * C])
            WqT = [wall[:, (0 * CT + t) * C:(0 * CT + t + 1) * C] for t in range(CT)]
            WkT = [wall[:, (1 * CT + t) * C:(1 * CT + t + 1) * C] for t in range(CT)]
            Wv = [wall[:, (2 * CT + t) * C:(2 * CT + t + 1) * C] for t in range(CT)]
            WoT = [wall[:, (3 * CT + t) * C:(3 * CT + t + 1) * C] for t in range(CT)]

            # f_n natural layout for phase 3: streams in during phase 2
            fnr_res = []
            for ch in range(NCH):
                fnr = fnpool.tile([P, BLK], BF16, tag=f"fn{ch}", name=f"fn{ch}")
                nc.sync.dma_start(fnr[:], fn_d[:, ch * BLK:(ch + 1) * BLK])
                fnr_res.append(fnr)

            # ---------- phase 2: logits, softmax, value mixing ------------
            for at in range(CT):
                if at % 2 == 0:
                    nc.vector.tensor_copy(G_sb[at][:], g_ps[at][:])
                else:
                    nc.scalar.activation(G_sb[at][:], g_ps[at][:], CP)

            # T2T[b, (h,i)] = sum_a G[a, b] * WqT[a, (h,i)]
            T2T_sb = []
            for bt in range(CT):
                ps = work.tile([P, C], F32, tag="wk1", name="t2tps")
                for at in range(CT):
                    nc.tensor.matmul(
                        ps[:],
                        G_sb[at][:, bt * P:(bt + 1) * P],
                        WqT[at][:],
                        start=(at == 0),
                        stop=(at == CT - 1),
                    )
                t = small.tile([P, C], BF16, tag=f"T2T_{bt}")
                if bt % 2 == 0:
                    nc.vector.tensor_copy(t[:], ps[:])
                else:
                    nc.scalar.activation(t[:], ps[:], CP)
                T2T_sb.append(t)

            # Diagonal head-pair tiles of D^T, stacked into two [128, 256]
            # PSUM tiles (split so the first half's exp doesn't wait on the
            # second half's matmuls -- psum deps are tile-granular)
            dps = [
                work.tile([P, 256], F32, tag="wk0", name=f"dps{hf}")
                for hf in range(2)
            ]
            for jt in range(CT):
                sl = slice(jt * P, (jt + 1) * P)
                dsl = slice((jt % 2) * P, (jt % 2 + 1) * P)
                for bt in range(CT):
                    nc.tensor.matmul(
                        dps[jt // 2][:, dsl], WkT[bt][:, sl], T2T_sb[bt][:, sl],
                        start=(bt == 0), stop=(bt == CT - 1),
                    )
            # E^T = exp(scale * D^T), cross-head blocks zeroed; per-half
            # exp/mask/rowsum/recip pipelines against the other half
            ew = small.tile([P, C], BF16, tag="ew")
            invw = small.tile([P, 8], F32, tag="invw")
            for hf in range(2):
                hs = slice(hf * 256, (hf + 1) * 256)
                etmp = small.tile([P, 256], F32, tag=f"etmp{hf}")
                nc.scalar.activation(etmp[:], dps[hf][:], EXP, scale=SCALE)
                nc.vector.tensor_mul(ew[:, hs], etmp[:], dmaskw[:, hs])
                rps = work.tile([P, 4], F32, tag="wk1", name=f"rps{hf}")
                for it in (2 * hf, 2 * hf + 1):
                    nc.tensor.matmul(rps[:, (it % 2) * 2:(it % 2) * 2 + 2],
                                     ew[:, it * P:(it + 1) * P], ones2w[:],
                                     start=True, stop=True)
                nc.vector.reciprocal(invw[:, hf * 4:(hf + 1) * 4], rps[:])

            # SE_h = E_h @ Wv_h. The softmax division is folded into Wout
            # instead of SE (rows of Wout^T are e=(h,i)-indexed, same as SE
            # rows): M = sum_e SE[e,:] * inv[e] * WoT[e,:] -- the row-scale
            # of WoT runs parallel to the SE casts on the other engine.
            SE_sb = []
            WoTs = []
            for it in range(CT):
                seps = gacc.tile([P, C], F32, tag=f"g{it}", name=f"seps{it}")
                nc.tensor.matmul(seps[:], ew[:, it * P:(it + 1) * P],
                                 Wv[it][:], start=True, stop=True)
                se_t = small.tile([P, C], BF16, tag=f"SE{it}")
                ws_t = small.tile([P, C], BF16, tag=f"WoTs{it}")
                inv_ap = invw[:, it * 2:it * 2 + 1]
                if it % 2 == 0:
                    nc.vector.tensor_copy(se_t[:], seps[:])
                    nc.scalar.activation(ws_t[:], WoT[it][:], CP, scale=inv_ap)
                else:
                    nc.scalar.activation(se_t[:], seps[:], CP)
                    nc.vector.tensor_scalar_mul(ws_t[:], WoT[it][:], inv_ap)
                SE_sb.append(se_t)
                WoTs.append(ws_t)

            # M^T[c, o] = sum_e SE[e][:, c] * WoTs[e][:, o]; et-major so each
            # accumulation step runs as soon as its SE/WoTs pair is ready
            mt_ps = [
                gacc.tile([P, C], F32, tag=f"g{ct}", name=f"mtps{ct}")
                for ct in range(CT)
            ]
            for et in range(CT):
                for ct in range(CT):
                    nc.tensor.matmul(
                        mt_ps[ct][:],
                        SE_sb[et][:, ct * P:(ct + 1) * P],
                        WoTs[et][:],
                        start=(et == 0),
                        stop=(et == CT - 1),
                    )
            MT_sb = []
            for ct in range(CT):
                t = small.tile([P, C], BF16, tag=f"MT{ct}")
                if ct % 2 == 0:
                    nc.vector.tensor_copy(t[:], mt_ps[ct][:])
                else:
                    nc.scalar.activation(t[:], mt_ps[ct][:], CP)
                MT_sb.append(t)

            # ---------- phase 3: out = M @ f_n + bout ----------------------
            # per-[128,512]-tile bias-add + store so the out stream drains
            # continuously (DVE/ACT alternate)
            # out kicks cost ~650ns each serialized on the sync engine, so
            # chunks 0-6 use one kick per ot-PAIR (2/chunk); the last chunk
            # uses pair + two singles so the final add->kick->transfer chain
            # after the very last matmul is as short as possible
            for ch in range(NCH):
                fnr = fnr_res[ch]
                last = ch == NCH - 1
                opair = None
                for ot in range(CT):
                    ps = gacc.tile([P, 512], F32, tag=f"g{ot}", name=f"ops{ot}")
                    for ct in range(CT):
                        nc.tensor.matmul(
                            ps[:],
                            MT_sb[ct][:, ot * P:(ot + 1) * P],
                            fnr[:, ct * 512:(ct + 1) * 512],
                            start=(ct == 0),
                            stop=(ct == CT - 1),
                        )
                    base = ch * BLK + ot * 512
                    if not last:
                        o = outst.tile([P, 512], F32, tag=f"o{ot}")
                        if ot % 2 == 0:
                            nc.vector.tensor_scalar_add(o[:], ps[:],
                                                        bout_sb[ot])
                        else:
                            nc.scalar.activation(o[:], ps[:], IDENT_FN,
                                                 bias=bout_sb[ot])
                        nc.sync.dma_start(out_d[:, base:base + 512], o[:])
                    elif ot < 2:
                        if ot == 0:
                            opair = outst.tile([P, 1024], F32, tag="opL")
                        osl = opair[:, ot * 512:(ot + 1) * 512]
                        if ot % 2 == 0:
                            nc.vector.tensor_scalar_add(osl, ps[:],
                                                        bout_sb[ot])
                        else:
                            nc.scalar.activation(osl, ps[:], IDENT_FN,
                                                 bias=bout_sb[ot])
                        if ot == 1:
                            nc.sync.dma_start(
                                out_d[:, base - 512:base + 512], opair[:]
                            )
                    else:
                        o = outst.tile([P, 512], F32, tag=f"oL{ot}")
                        if ot == 3:
                            # very last tile: halves on both engines in
                            # parallel to shorten the drain chain
                            nc.vector.tensor_scalar_add(o[:, 0:256],
                                                        ps[:, 0:256],
                                                        bout_sb[ot])
                            nc.scalar.activation(o[:, 256:512],
                                                 ps[:, 256:512], IDENT_FN,
                                                 bias=bout_sb[ot])
                        else:
                            nc.vector.tensor_scalar_add(o[:], ps[:],
                                                        bout_sb[ot])
                        nc.sync.dma_start(out_d[:, base:base + 512], o[:])

    nc.compile()
    return nc


def _get_nc():
    global _CACHED_NC
    if _CACHED_NC is None:
        _CACHED_NC = _build()
    return _CACHED_NC


def _get_runner():
    """Memoized PJRT runner: jax.jit-compiled once, reused across kernel()
    calls (run_bass_kernel_spmd rebuilds the jit closure every call, which
    forces a ~minute-long recompile)."""
    global _CACHED_RUNNER
    if _CACHED_RUNNER is not None:
        return _CACHED_RUNNER

    import jax
    from jax.sharding import Mesh, PartitionSpec
    from jax.experimental.shard_map import shard_map
    import concourse.mybir as mybir_
    from concourse.bass2jax import (
        _bass_exec_p,
        install_neuronx_cc_hook,
        partition_id_tensor,
    )

    nc = _get_nc()
    install_neuronx_cc_hook()

    partition_name = (
        nc.partition_id_tensor.name if nc.partition_id_tensor else None
    )
    in_names = []
    out_names = []
    out_avals = []
    out_shapes = []
    for alloc in nc.m.functions[0].allocations:
        if not isinstance(alloc, mybir_.MemoryLocationSet):
            continue
        name = alloc.memorylocations[0].name
        if alloc.kind == "ExternalInput":
            if name != partition_name:
                in_names.append(name)
        elif alloc.kind == "ExternalOutput":
            shape = tuple(alloc.tensor_shape)
            dtype = mybir_.dt.np(alloc.dtype)
            out_names.append(name)
            out_avals.append(jax.core.ShapedArray(shape, dtype))
            out_shapes.append((shape, dtype))
    n_params = len(in_names)
    n_outs = len(out_names)
    all_names = tuple(in_names + out_names)
    if partition_name is not None:
        all_names = all_names + (partition_name,)
    donate = tuple(range(n_params, n_params + n_outs))

    def _body(*args):
        operands = list(args)
        if partition_name is not None:
            operands.append(partition_id_tensor())
        outs = _bass_exec_p.bind(
            *operands,
            out_avals=tuple(out_avals),
            in_names=all_names,
            out_names=tuple(out_names),
            lowering_input_output_aliases=(),
            sim_require_finite=True,
            sim_require_nnan=True,
            nc=nc,
        )
        return tuple(outs)

    devices = jax.devices()[:B]
    mesh = Mesh(np.asarray(devices), ("core",))
    sharded = jax.jit(
        shard_map(
            _body,
            mesh=mesh,
            in_specs=(PartitionSpec("core"),) * (n_params + n_outs),
            out_specs=(PartitionSpec("core"),) * n_outs,
            check_rep=False,
        ),
        donate_argnums=donate,
        keep_unused=True,
    )

    def run(in_maps):
        concat_in = [
            np.concatenate([np.asarray(m[k]) for m in in_maps], axis=0)
            for k in in_names
        ]
        concat_zeros = [
            np.zeros((B * s[0], *s[1:]), dt) for (s, dt) in out_shapes
        ]
        out_arrs = sharded(*concat_in, *concat_zeros)
        return [
            {
                k: np.asarray(out_arrs[i]).reshape(B, *out_shapes[i][0])[c]
                for i, k in enumerate(out_names)
            }
            for c in range(B)
        ]

    _CACHED_RUNNER = run
    return run


def _marshal(f_m, f_n, Wq, Wkv, Wout, bout):
    f32 = np.float32
    b = f_m.shape[0]
    # x2[p, (ch, su, c)] = x[c, (ch*4+su)*128 + p]  (pre-transposed [n, c])
    fm = np.asarray(f_m, f32).reshape(b, C, NSUB, P)
    fm2 = np.ascontiguousarray(fm.transpose(0, 3, 2, 1).reshape(b, P, NCH * BLK))
    fm2 = fm2.astype(_BF)
    fnt = np.asarray(f_n, f32).reshape(b, C, NSUB, P)
    fnt2 = np.ascontiguousarray(fnt.transpose(0, 3, 2, 1).reshape(b, P, NCH * BLK))
    fnt2 = fnt2.astype(_BF)
    # fn2[p, (ch, ct, n')] = f_n[ct*128 + p, ch*512 + n']  (natural)
    fn = np.asarray(f_n, f32).reshape(b, CT, P, NCH, 512)
    fn2 = np.ascontiguousarray(fn.transpose(0, 2, 3, 1, 4).reshape(b, P, NCH * BLK))
    fn2 = fn2.astype(_BF)

    Wq = np.asarray(Wq, f32)
    Wkv = np.asarray(Wkv, f32)
    Wout = np.asarray(Wout, f32)
    w2 = np.empty((P, 16 * C), f32)
    for wi, W in enumerate([Wq.T, Wkv[:C].T, Wkv[C:], Wout.T]):
        for t in range(CT):
            w2[:, (wi * CT + t) * C:(wi * CT + t + 1) * C] = W[t * P:(t + 1) * P, :]
    w2 = w2.astype(_BF)

    bout = np.asarray(bout, f32)
    cstf = np.zeros((P, 516), f32)
    cstf[:, 0:512] = _DMASKW
    cstf[:, 512:516] = bout.reshape(CT, P).T
    return fm2, fnt2, fn2, w2, cstf


def kernel(f_m, f_n, Wq, Wkv, Wout, bout, trace=False):
    b, c, h, w = f_m.shape
    nc = _get_nc()
    fm2, fnt2, fn2, w2, cstf = _marshal(f_m, f_n, Wq, Wkv, Wout, bout)
    in_maps = [
        {
            "fm2": fm2[i],
            "fnt2": fnt2[i],
            "fn2": fn2[i],
            "w2": w2,
            "cstf": cstf,
        }
        for i in range(b)
    ]
    if trace:
        res = run_bass_kernel_spmd(
            nc, in_maps, core_ids=list(range(B)), trace=True
        )
        kernel.last_results = res
        results = res.results
    else:
        results = _get_runner()(in_maps)
    # out[p, (ch, ot, n')] -> out_full[ot*128+p, ch*512+n']
    outs = []
    for r in results:
        o = r["out"].reshape(P, NCH, CT, 512)
        outs.append(
            np.ascontiguousarray(o.transpose(2, 0, 1, 3)).reshape(c, h, w)
        )
    return np.stack(outs)
